# revision 3
# baseline (speedup 1.0000x reference)
"""GCN v2 on 8 TRN2 cores: 4-queue dma_gather, feature-major h, swapped
one-hot matmuls, pipelined per-slice AllGathers.

Dataflow per conv layer (per core):
  - table tbl[buf] [ROWS, 128] bf16 in DRAM: row srow(n) = cb*NPAD + w*128 + p
    holds t(n) = dis(n) * (h(n) @ W) in cols 0:64 (cols 64:128 garbage).
  - gather: per (dst window w, src half sh) bucket, chunks of 128 edges;
    dma_gather (1024 idx/call, queues round-robin 0-3) pulls t[src] rows.
  - S chunk [128 e, 128 d] = is_equal(iota, dstloc) on DVE.
  - matmul per chunk: psum[64 h, 128 d] += msg[:, slot*128:+64]^T . S
  - self term: psum += tpad_window^T via matmul with identity rhs.
  - drain per group of 8 windows: h_fm = relu(psum * disP)  (feature-major)
  - transform: psum_nm[128 n, 64] = h_fm_window^T . W2; tpad = psum * disn;
    DMA tpad slice -> own_slice -> per-slice AllGather -> tbl[other buf].
  - layer 5: transpose h_fm windows -> h_nm, one-hot pooling matmuls,
    partial per-graph sums out; host divides by counts and applies Wl.
"""

import math
import sys

sys.path.insert(0, "/opt/trn_rl_repo")

import numpy as np
import ml_dtypes

import concourse.bass as bass
import concourse.mybir as mybir
import concourse.tile as tile
from concourse import bacc
from concourse.masks import make_identity

BF16 = mybir.dt.bfloat16
F32 = mybir.dt.float32
I16 = mybir.dt.int16
ALU = mybir.AluOpType

NP_BF16 = ml_dtypes.bfloat16

CALLCH = 8      # chunks per dma_gather call (1024 idx HW limit)
WGRP = 8        # windows per psum group ([64, 8*128] f32 = 2 banks)
NQ = 4          # SWDGE queues for gather round-robin


def prep(x, W1, b1, W2, b2, Wl, bl, edge_index, batch, C, G):
    x = np.asarray(x, np.float32)
    W1 = np.asarray(W1, np.float32); b1 = np.asarray(b1, np.float32)
    W2 = np.asarray(W2, np.float32); b2 = np.asarray(b2, np.float32)
    Wl = np.asarray(Wl, np.float32); bl = np.asarray(bl, np.float32)
    edge_index = np.asarray(edge_index, np.int64)
    batch = np.asarray(batch, np.int64)
    assert not np.any(b1) and not np.any(b2), "zero biases assumed"

    N, F = x.shape
    E = edge_index.shape[1]
    H = W1.shape[1]
    assert N % C == 0 and C % 2 == 0
    NPC = N // C
    W = math.ceil(NPC / 128)
    NPAD = W * 128
    ROWS = C * NPAD
    HR = (C // 2) * NPAD
    assert HR <= 32768, HR
    NG = math.ceil(W / WGRP)

    src, dst = edge_index[0], edge_index[1]
    deg = 1.0 + np.bincount(dst, minlength=N).astype(np.float32)
    dis = 1.0 / np.sqrt(deg)

    n = np.arange(N)
    cb = n // NPC
    lp = n % NPC
    w_ = lp // 128
    p_ = lp % 128
    # slice-major table rows: [slice][core][window-in-slice][p]; each
    # per-slice AllGather output is then contiguous.
    SLICE_W = 2 * WGRP
    NSLICE = math.ceil(W / SLICE_W)
    Ls = np.array([min(SLICE_W, W - SLICE_W * s) for s in range(NSLICE)])
    s_ = w_ // SLICE_W
    srow = (C * 128 * SLICE_W * s_ + cb * Ls[s_] * 128
            + (w_ - SLICE_W * s_) * 128 + p_)
    HB = 32768                     # int16-addressable row window
    BASE1 = ROWS - HB

    # --- edge bucketing: (core, src-half, dst window) ---------------------
    # rows in [BASE1, HB) are addressable from both gather bases; assign
    # those "flex" edges per-core to whichever half minimizes chunk padding.
    ecore = dst // NPC
    edl = dst % NPC
    ew = edl // 128
    edp = edl % 128
    srcrow = srow[src]
    cls = (srcrow >= BASE1).astype(np.int64) + (srcrow >= HB).astype(np.int64)
    key3 = (ecore * W + ew) * 3 + cls
    cnt3 = np.bincount(key3, minlength=C * W * 3).reshape(C, W, 3)
    n0, nf, n1 = cnt3[:, :, 0], cnt3[:, :, 1], cnt3[:, :, 2]
    Kb = np.zeros((2, W), np.int64)
    fvec = np.zeros((C, W), np.int64)
    for w in range(W):
        k0lo = math.ceil(n0[:, w].max() / 128)
        k0hi = math.ceil((n0[:, w] + nf[:, w]).max() / 128)
        best = None
        for k0 in range(k0lo, k0hi + 1):
            f = np.minimum(nf[:, w], np.maximum(k0 * 128 - n0[:, w], 0))
            k1 = math.ceil((n1[:, w] + nf[:, w] - f).max() / 128)
            if best is None or k0 + k1 < best[0] + best[1]:
                best = (k0, k1, f.copy())
        Kb[0, w], Kb[1, w], fvec[:, w] = best
        if Kb[:, w].sum() == 0:
            Kb[0, w] = 1
    # per-edge half: must0/must1 fixed; first f flex edges (per core,w) -> 0
    order0 = np.lexsort((cls, ew, ecore))
    gkey = (ecore * W + ew) * 3 + cls
    sg = gkey[order0]
    uniq, first = np.unique(sg, return_index=True)
    rank3 = np.arange(E) - first[np.searchsorted(uniq, sg)]
    esh = np.zeros(E, np.int64)
    cls_s = cls[order0]
    esh_s = np.where(cls_s == 2, 1,
                     np.where(cls_s == 0, 0,
                              (rank3 >= fvec[ecore[order0], ew[order0]])
                              .astype(np.int64)))
    esh[order0] = esh_s

    # chunk order: (group, src-half, window)
    chunk_w, chunk_sh = [], []
    boff = np.zeros((2, W), np.int64)
    calls = []          # (sh, c_lo, c_hi)
    for g in range(NG):
        wlo, whi = g * WGRP, min((g + 1) * WGRP, W)
        for sh in (0, 1):
            lo = len(chunk_w)
            for w in range(wlo, whi):
                boff[sh, w] = len(chunk_w)
                for _ in range(int(Kb[sh, w])):
                    chunk_w.append(w); chunk_sh.append(sh)
            c0 = lo
            while c0 < len(chunk_w):
                c1 = min(c0 + CALLCH, len(chunk_w))
                calls.append((sh, c0, c1))
                c0 = c1
    NCHUNK = len(chunk_w)
    chunk_w = np.array(chunk_w); chunk_sh = np.array(chunk_sh)

    # --- per-core edge payloads ------------------------------------------
    idx_all = np.zeros((C, NCHUNK * 128), np.int16)
    dl_all = np.full((C, NCHUNK * 128), 400.0, np.float32)
    for c in range(C):
        m = ecore == c
        es, ish, iw, idl = src[m], esh[m], ew[m], edp[m]
        order = np.lexsort((iw, ish))
        es, ish, iw, idl = es[order], ish[order], iw[order], idl[order]
        key = ish * W + iw
        uniq, first = np.unique(key, return_index=True)
        ranks = np.arange(len(key)) - first[np.searchsorted(uniq, key)]
        pos = boff[ish, iw] * 128 + ranks
        idx_all[c, pos] = (srow[es] - ish * BASE1).astype(np.int16)
        dl_all[c, pos] = idl

    idx16 = np.zeros((C, 128, NCHUNK * 8), np.int16)
    for c in range(C):
        wrapped = idx_all[c].reshape(NCHUNK * 8, 16).T
        idx16[c] = np.tile(wrapped, (8, 1))
    dstloc = np.zeros((C, 128, NCHUNK), NP_BF16)
    for c in range(C):
        dstloc[c] = dl_all[c].reshape(NCHUNK, 128).T.astype(NP_BF16)

    # --- node-side tensors ------------------------------------------------
    xfm = np.zeros((F, ROWS), np.float32)
    xfm[:, cb * NPAD + lp] = x.T      # block-linear cols (not table srow)
    xfm = xfm.astype(NP_BF16)

    disALL = np.zeros((128, C * W), np.float32)
    disALL[p_, cb * W + w_] = dis
    disn = np.zeros((C, 128, W), np.float32)
    disP = np.zeros((C, 128, NPAD), np.float32)
    for c in range(C):
        sl = slice(c * NPC, (c + 1) * NPC)
        disn[c][p_[sl], w_[sl]] = dis[sl]
        disP[c][:, :NPC] = dis[sl][None, :]

    # --- pooling ----------------------------------------------------------
    BLK = math.ceil(G / 128) + 3
    wk = [int(batch[c * NPC]) // 128 for c in range(C)]
    glocal = np.full((C, 128, W), 1.0e4, np.float32)
    for c in range(C):
        sl = slice(c * NPC, (c + 1) * NPC)
        gl = batch[sl] - 128 * wk[c]
        assert gl.min() >= 0 and gl.max() < 384, (c, gl.min(), gl.max())
        glocal[c][p_[sl], w_[sl]] = gl

    iota_rep = np.tile(np.arange(128, dtype=np.float32), CALLCH)
    iota_rep = np.tile(iota_rep.reshape(1, -1), (128, 1)).astype(NP_BF16)
    iotaP = np.tile(np.arange(384, dtype=np.float32).reshape(1, 384), (128, 1))

    meta = dict(
        N=N, F=F, H=H, E=E, G=G, C=C, NPC=NPC, W=W, NPAD=NPAD, ROWS=ROWS,
        HR=HR, HB=HB, BASE1=BASE1, SLICE_W=SLICE_W, Ls=Ls,
        NG=NG, NCHUNK=NCHUNK, chunk_w=chunk_w, chunk_sh=chunk_sh,
        boff=boff, Kb=Kb, calls=calls, wk=wk, BLK=BLK,
        blv=float(bl.reshape(-1)[0]),
    )
    shared = dict(
        xfm=xfm,
        disALL=disALL,
        W1sb=W1.astype(NP_BF16),
        W2sb=W2.astype(NP_BF16),
        iota_rep=iota_rep,
        iotaP=iotaP,
    )
    in_maps = []
    for c in range(C):
        m = dict(shared)
        m["xown"] = np.ascontiguousarray(xfm[:, c * NPAD:(c + 1) * NPAD])
        m["idx16"] = idx16[c]
        m["dstloc"] = dstloc[c]
        m["disn"] = disn[c]
        m["disP"] = disP[c][:H]
        m["glocal"] = glocal[c]
        in_maps.append(m)
    return meta, in_maps


# ---------------------------------------------------------------------------
# numpy simulation of the device dataflow (layout/bucketing validation)
# ---------------------------------------------------------------------------

def sim_global(meta, in_maps, inputs, C, G):
    """Global-view simulation: one table pass per layer, all cores."""
    bf = lambda a: a.astype(NP_BF16).astype(np.float32)
    W_, NPAD, ROWS = meta["W"], meta["NPAD"], meta["ROWS"]
    BASE1, SLICE_W, Ls = meta["BASE1"], meta["SLICE_W"], meta["Ls"]
    H, NCHUNK, NPC = meta["H"], meta["NCHUNK"], meta["NPC"]
    chunk_w, chunk_sh = meta["chunk_w"], meta["chunk_sh"]
    xfm = np.asarray(in_maps[0]["xfm"], np.float32)
    W1 = np.asarray(in_maps[0]["W1sb"], np.float32)
    W2 = np.asarray(in_maps[0]["W2sb"], np.float32)
    disALL = in_maps[0]["disALL"]

    def rowbase(cblk, w):
        s = w // SLICE_W
        return (C * 128 * SLICE_W * s + cblk * int(Ls[s]) * 128
                + (w - SLICE_W * s) * 128)

    # conv1 table
    tbl = np.zeros((ROWS, 128), np.float32)
    for cblk in range(C):
        xb = xfm[:, cblk * NPAD:(cblk + 1) * NPAD]
        for w in range(W_):
            ps = xb[:, w * 128:(w + 1) * 128].T @ W1
            sc = disALL[:, cblk * W_ + w]
            rows = rowbase(cblk, w) + np.arange(128)
            tbl[rows, :H] = bf(ps * sc[:, None])

    idxs_all, dl_all = [], []
    for c in range(C):
        idx16 = in_maps[c]["idx16"]
        idxs_all.append(idx16[:16].T.reshape(-1).astype(np.int64))
        dl_all.append(np.asarray(in_maps[c]["dstloc"], np.float32))

    h_fm = [None] * C
    for layer in range(1, 6):
        newh = []
        for c in range(C):
            im = in_maps[c]
            disP = im["disP"]
            agg = np.zeros((H, NPAD), np.float32)
            for ci in range(NCHUNK):
                w = int(chunk_w[ci]); sh = int(chunk_sh[ci])
                rows = idxs_all[c][ci * 128:(ci + 1) * 128] + sh * BASE1
                msg = tbl[rows, :H]
                dl = dl_all[c][:, ci]
                S = (np.arange(128)[None, :] == dl[:, None]).astype(np.float32)
                agg[:, w * 128:(w + 1) * 128] += msg.T @ S
            # self term from tbl own rows
            for w in range(W_):
                rows = rowbase(c, w) + np.arange(128)
                agg[:, w * 128:(w + 1) * 128] += tbl[rows, :H].T
            newh.append(bf(np.maximum(agg * disP, 0.0)))
        h_fm = newh
        if layer == 5:
            break
        # transform + exchange
        newtbl = np.zeros((ROWS, 128), np.float32)
        for c in range(C):
            disn = in_maps[c]["disn"]
            for w in range(W_):
                ps = h_fm[c][:, w * 128:(w + 1) * 128].T @ W2
                t = bf(ps * disn[:, w][:, None])
                rows = rowbase(c, w) + np.arange(128)
                newtbl[rows, :H] = t
        tbl = newtbl

    parts = []
    for c in range(C):
        glocal = in_maps[c]["glocal"]
        h_nm = np.zeros((128, W_ * H), np.float32)
        for w in range(W_):
            h_nm[:, w * H:(w + 1) * H] = h_fm[c][:, w * 128:(w + 1) * 128].T
        pps = np.zeros((128, 3 * H), np.float32)
        for blk in range(3):
            for w in range(W_):
                SG = (glocal[:, w][:, None] ==
                      (blk * 128 + np.arange(128))[None, :]).astype(np.float32)
                pps[:, blk * H:(blk + 1) * H] += \
                    SG.T @ bf(h_nm[:, w * H:(w + 1) * H])
        parts.append(pps)
    return parts


def _ap3(ap, pattern, offset=None):
    return bass.AP(ap.tensor, ap.offset if offset is None else offset, pattern)


def build(nc, meta, debug=False):
    F, H, C = meta["F"], meta["H"], meta["C"]
    W, NPAD, ROWS = meta["W"], meta["NPAD"], meta["ROWS"]
    HB, BASE1, SLICE_W, Ls = (meta["HB"], meta["BASE1"], meta["SLICE_W"],
                              meta["Ls"])
    NG, NCHUNK = meta["NG"], meta["NCHUNK"]
    chunk_w, chunk_sh = meta["chunk_w"], meta["chunk_sh"]
    calls = meta["calls"]
    rg = [list(range(C))]

    def rowbase(cblk, w):
        s = w // SLICE_W
        return (C * 128 * SLICE_W * s + cblk * int(Ls[s]) * 128
                + (w - SLICE_W * s) * 128)

    # host-side emission plan: per chunk -> first/last of its psum bank
    bank_of_chunk = chunk_w // 4
    first_chunk_of_w = {}
    last_chunk_of_bank = {}
    for ci in range(NCHUNK):
        w = int(chunk_w[ci])
        if w not in first_chunk_of_w:
            first_chunk_of_w[w] = ci
        last_chunk_of_bank[int(bank_of_chunk[ci])] = ci
    last_chunk_of_group = {}
    for ci in range(NCHUNK):
        last_chunk_of_group[int(chunk_w[ci]) // WGRP] = ci
    drain_after = {v: k for k, v in last_chunk_of_group.items()}
    stop_of_bank = {v: k for k, v in last_chunk_of_bank.items()}

    # exchange slices: after these groups, AllGather windows [wlo, whi)
    slice_after = {}
    acc = []
    for g in range(NG):
        acc.append(g)
        if g % 2 == 1 or g == NG - 1:
            wlo = acc[0] * WGRP
            whi = min((acc[-1] + 1) * WGRP, W)
            slice_after[g] = (wlo, whi)
            acc = []

    # external inputs
    xfm_e = nc.dram_tensor("xfm", [F, ROWS], BF16, kind="ExternalInput")
    xown_e = nc.dram_tensor("xown", [F, NPAD], BF16, kind="ExternalInput")
    disALL_e = nc.dram_tensor("disALL", [128, C * W], F32, kind="ExternalInput")
    W1_e = nc.dram_tensor("W1sb", [F, H], BF16, kind="ExternalInput")
    W2_e = nc.dram_tensor("W2sb", [H, H], BF16, kind="ExternalInput")
    iota_e = nc.dram_tensor("iota_rep", [128, CALLCH * 128], BF16,
                            kind="ExternalInput")
    iotaP_e = nc.dram_tensor("iotaP", [128, 384], F32, kind="ExternalInput")
    idx_e = nc.dram_tensor("idx16", [128, NCHUNK * 8], I16, kind="ExternalInput")
    dstloc_e = nc.dram_tensor("dstloc", [128, NCHUNK], BF16, kind="ExternalInput")
    disn_e = nc.dram_tensor("disn", [128, W], F32, kind="ExternalInput")
    disP_e = nc.dram_tensor("disP", [H, NPAD], F32, kind="ExternalInput")
    glocal_e = nc.dram_tensor("glocal", [128, W], F32, kind="ExternalInput")
    out_e = nc.dram_tensor("out", [128, 3 * H], F32, kind="ExternalOutput")
    dbg = {}
    if debug:
        dbg["tbl1"] = nc.dram_tensor("dbg_tbl1", [ROWS, 128], BF16,
                                     kind="ExternalOutput")
        for l in range(1, 6):
            dbg[f"h{l}"] = nc.dram_tensor(f"dbg_h{l}", [H, NPAD], BF16,
                                          kind="ExternalOutput")

    shared = "Shared" if C > 4 else "Local"
    tbl = [nc.dram_tensor(f"tbl{i}", [ROWS, 128], BF16, addr_space=shared)
           for i in range(2)]
    own_slice = [nc.dram_tensor(f"own_slice{i}", [NPAD, 128], BF16)
                 for i in range(2)]

    with tile.TileContext(nc) as tc:
        from contextlib import ExitStack
        with ExitStack() as ctx:
            cpool = ctx.enter_context(tc.tile_pool(name="const", bufs=1))
            mpool = ctx.enter_context(tc.tile_pool(name="msg", bufs=12))
            spool = ctx.enter_context(tc.tile_pool(name="s", bufs=6))
            dpool = ctx.enter_context(tc.tile_pool(name="dexp", bufs=4))
            tpool = ctx.enter_context(tc.tile_pool(name="tmp", bufs=3))
            agg_ps = ctx.enter_context(
                tc.tile_pool(name="aggps", bufs=2, space="PSUM"))
            tf_ps = ctx.enter_context(
                tc.tile_pool(name="tfps", bufs=2, space="PSUM"))
            pl_ps = ctx.enter_context(
                tc.tile_pool(name="plps", bufs=1, space="PSUM"))
            tp_ps = ctx.enter_context(
                tc.tile_pool(name="tpps", bufs=1, space="PSUM"))

            def load(name, ext, shape, dt):
                t = cpool.tile(shape, dt, tag=name)
                nc.sync.dma_start(t[:], ext.ap())
                return t

            W1sb = load("W1", W1_e, [F, H], BF16)
            W2sb = load("W2", W2_e, [H, H], BF16)
            iota_rep = load("iota", iota_e, [128, CALLCH * 128], BF16)
            iotaP = load("iotaP", iotaP_e, [128, 384], F32)
            idxsb = load("idx", idx_e, [128, NCHUNK * 8], I16)
            dstloc = load("dstloc", dstloc_e, [128, NCHUNK], BF16)
            disn = load("disn", disn_e, [128, W], F32)
            disP = load("disP", disP_e, [H, NPAD], F32)
            disALL = load("disALL", disALL_e, [128, C * W], F32)
            glocal = load("glocal", glocal_e, [128, W], F32)
            ident = cpool.tile([128, 128], BF16, tag="ident")
            make_identity(nc, ident[:])

            h_fm = cpool.tile([H, NPAD], BF16, tag="hfm")
            tpads = [cpool.tile([128, W * 128], BF16, tag=f"tpad{i}",
                                name=f"tpad{i}") for i in range(2)]
            h_nm = cpool.tile([128, W * H], BF16, tag="hnm")

            def sc_bc(t, lo, n, inner, nparts=128):
                a = t[:]
                return _ap3(a, [[a.ap[0][0], nparts], [1, n], [0, inner]],
                            a.offset + lo)

            # ---------------- conv1: local full-table build ----------------
            qctr = [0]

            def conv1(xpool):
                # own-block transform first -> tpad[0] (gates layer-1 self
                # matmuls, which gate every psum bank start)
                xo = xpool.tile([F, NPAD], BF16, tag="xfm", name="xo")
                nc.sync.dma_start(xo[:], xown_e.ap())
                for wlo in range(0, W, 8):
                    nb = min(8, W - wlo)
                    ps = tf_ps.tile([128, 8 * H], F32, tag="tf")
                    for i in range(nb):
                        col = (wlo + i) * 128
                        nc.tensor.matmul(
                            ps[:, i * H:(i + 1) * H],
                            lhsT=xo[:, col:col + 128],
                            rhs=W1sb[:],
                            start=True, stop=True, skip_group_check=True)
                    ps3 = _ap3(ps[:], [[ps[:].ap[0][0], 128], [H, nb], [1, H]])
                    tp = tpads[0][:]
                    tp3 = _ap3(tp, [[tp.ap[0][0], 128], [128, nb], [1, H]],
                               tp.offset + wlo * 128)
                    nc.vector.tensor_tensor(
                        tp3, ps3, sc_bc(disn, wlo, nb, H), op=ALU.mult)
                for cblk in range(C):
                    xblk = xpool.tile([F, NPAD], BF16, tag="xfm")
                    nc.sync.dma_start(
                        xblk[:], xfm_e.ap()[:, cblk * NPAD:(cblk + 1) * NPAD])
                    for wlo in range(0, W, 8):
                        nb = min(8, W - wlo)
                        ps = tf_ps.tile([128, 8 * H], F32, tag="tf")
                        for i in range(nb):
                            col = (wlo + i) * 128
                            nc.tensor.matmul(
                                ps[:, i * H:(i + 1) * H],
                                lhsT=xblk[:, col:col + 128],
                                rhs=W1sb[:],
                                start=True, stop=True, skip_group_check=True)
                        # full 256B rows (upper 64 cols junk, never read):
                        # halves DMA descriptor count vs 128B writes
                        pd = tpool.tile([128, 8 * 128], BF16, tag="pd")
                        ps3 = _ap3(ps[:], [[ps[:].ap[0][0], 128], [H, nb], [1, H]])
                        pd3 = _ap3(pd[:], [[pd[:].ap[0][0], 128], [128, nb], [1, H]])
                        nc.vector.tensor_tensor(
                            pd3, ps3, sc_bc(disALL, cblk * W + wlo, nb, H),
                            op=ALU.mult)
                        dr = _ap3(tbl[0].ap(),
                                  [[128, 128], [128 * 128, nb], [1, 128]],
                                  rowbase(cblk, wlo) * 128)
                        sr = _ap3(pd[:], [[pd[:].ap[0][0], 128], [128, nb], [1, 128]])
                        nc.sync.dma_start(dr, sr)

            # ---------------- aggregation ----------------
            def aggregate(layer, after_group):
                cur = tbl[(layer - 1) % 2]
                tpad = tpads[(layer - 1) % 2]
                psg = {}
                bank_start = {}
                windows_seen = set()

                def emit_self(w, g):
                    bk = int(w // 4)
                    first = bk not in bank_start
                    wl = w - g * WGRP
                    mm = nc.tensor.matmul(
                        psg[g][:, wl * 128:(wl + 1) * 128],
                        lhsT=tpad[:, w * 128:w * 128 + H],
                        rhs=ident[:],
                        start=first, stop=False, skip_group_check=True)
                    if first:
                        bank_start[bk] = mm
                    else:
                        bass._add_dep_helper(mm.ins, bank_start[bk].ins,
                                             sync=False, reason="psum order")

                for (sh, c_lo, c_hi) in calls:
                    ncall = c_hi - c_lo
                    msg = mpool.tile([128, CALLCH * 128], BF16, tag="msg")
                    in_ap = bass.AP(cur.ap().tensor,
                                    cur.ap().offset + sh * BASE1 * 128,
                                    [[128, HB], [1, 128]])
                    nc.gpsimd.dma_gather(
                        out_ap=_ap3(msg[:], [[msg[:].ap[0][0], 128],
                                             [128, ncall], [1, 128]]),
                        in_ap=in_ap,
                        idxs_ap=idxsb[:, c_lo * 8:c_hi * 8],
                        num_idxs=ncall * 128,
                        num_idxs_reg=ncall * 128,
                        elem_size=128,
                        queue_num=qctr[0] % NQ)
                    qctr[0] += 1
                    # expand dstloc on the (idle) scalar engine so the DVE
                    # is_equal sees all stride-1 operands (2x perf mode)
                    dexp = dpool.tile([128, CALLCH * 128], BF16, tag="dexp")
                    dx3 = _ap3(dexp[:], [[dexp[:].ap[0][0], 128],
                                         [128, ncall], [1, 128]])
                    nc.scalar.copy(dx3, sc_bc(dstloc, c_lo, ncall, 128))
                    S = spool.tile([128, CALLCH * 128], BF16, tag="S")
                    S3 = _ap3(S[:], [[S[:].ap[0][0], 128], [128, ncall], [1, 128]])
                    io3 = _ap3(iota_rep[:],
                               [[iota_rep[:].ap[0][0], 128], [128, ncall], [1, 128]])
                    nc.vector.tensor_tensor(S3, io3, dx3, op=ALU.is_equal)
                    for j in range(ncall):
                        ci = c_lo + j
                        w = int(chunk_w[ci])
                        g = w // WGRP
                        if g not in psg:
                            psg[g] = agg_ps.tile([H, WGRP * 128], F32,
                                                 tag="agg",
                                                 name=f"agg_l{layer}_g{g}")
                        if w not in windows_seen:
                            windows_seen.add(w)
                            emit_self(w, g)
                        wl = w - g * WGRP
                        bk = int(bank_of_chunk[ci])
                        mm = nc.tensor.matmul(
                            psg[g][:, wl * 128:(wl + 1) * 128],
                            lhsT=msg[:, j * 128:j * 128 + H],
                            rhs=S[:, j * 128:(j + 1) * 128],
                            start=False,
                            stop=(stop_of_bank.get(ci) is not None),
                            skip_group_check=True)
                        bass._add_dep_helper(mm.ins, bank_start[bk].ins,
                                             sync=False, reason="psum order")
                        if ci in drain_after:
                            g_done = drain_after[ci]
                            drain(layer, g_done, psg.pop(g_done))
                            after_group(g_done)

            def drain(layer, g, ps):
                wlo = g * WGRP
                nb = min(WGRP, W - wlo)
                tmp = tpool.tile([H, WGRP * 128], F32, tag="dr")
                nc.vector.tensor_tensor(
                    tmp[:, :nb * 128], ps[:, :nb * 128],
                    disP[:, wlo * 128:(wlo + nb) * 128], op=ALU.mult)
                nc.vector.tensor_scalar(
                    h_fm[:, wlo * 128:(wlo + nb) * 128], tmp[:, :nb * 128],
                    0.0, None, op0=ALU.max)
                if debug:
                    nc.sync.dma_start(
                        dbg[f"h{layer}"].ap()[:, wlo * 128:(wlo + nb) * 128],
                        h_fm[:, wlo * 128:(wlo + nb) * 128])

            # ---------------- transform + exchange ----------------
            def make_transform(layer):
                nxt = tbl[layer % 2]
                osl = own_slice[layer % 2]
                tpad = tpads[layer % 2]

                def after_group(g):
                    wlo = g * WGRP
                    nb = min(WGRP, W - wlo)
                    ps = tf_ps.tile([128, 8 * H], F32, tag="tf",
                                    name=f"tf_l{layer}_g{g}")
                    for i in range(nb):
                        col = (wlo + i) * 128
                        nc.tensor.matmul(
                            ps[:, i * H:(i + 1) * H],
                            lhsT=_ap3(h_fm[:], [[h_fm[:].ap[0][0], H], [1, 128]],
                                      h_fm[:].offset + col),
                            rhs=W2sb[:],
                            start=True, stop=True, skip_group_check=True)
                    ps3 = _ap3(ps[:], [[ps[:].ap[0][0], 128], [H, nb], [1, H]])
                    tp = tpad[:]
                    tp3 = _ap3(tp, [[tp.ap[0][0], 128], [128, nb], [1, H]],
                               tp.offset + wlo * 128)
                    nc.vector.tensor_tensor(
                        tp3, ps3, sc_bc(disn, wlo, nb, H), op=ALU.mult)
                    # tpad slice -> own_slice rows [wlo*128, ...), full 256B
                    # rows (upper halves junk, never read)
                    dr = _ap3(osl.ap(), [[128, 128], [128 * 128, nb], [1, 128]],
                              wlo * 128 * 128)
                    sr = _ap3(tp, [[tp.ap[0][0], 128], [128, nb], [1, 128]],
                              tp.offset + wlo * 128)
                    nc.sync.dma_start(dr, sr)
                    if g in slice_after:
                        slo, shi = slice_after[g]
                        nwe = (shi - slo) * 128 * 128
                        nc.gpsimd.collective_compute(
                            "AllGather", ALU.bypass, replica_groups=rg,
                            ins=[bass.AP(osl.ap().tensor,
                                         osl.ap().offset + slo * 128 * 128,
                                         [[1, nwe]])],
                            outs=[bass.AP(nxt.ap().tensor,
                                          nxt.ap().offset + C * slo * 128 * 128,
                                          [[1, C * nwe]])])
                return after_group

            # ---------------- pooling (layer 5) ----------------
            pool_state = {}

            def pool_group(g):
                wlo = g * WGRP
                nb = min(WGRP, W - wlo)
                if "pps" not in pool_state:
                    pool_state["pps"] = pl_ps.tile([128, 3 * H], F32,
                                                   tag="pps", name="pps")
                pps = pool_state["pps"]
                # transpose h_fm windows -> h_nm (node-major)
                for i in range(nb):
                    w = wlo + i
                    tp = tp_ps.tile([128, H], F32, tag="tp", name=f"tp{w}")
                    ident64 = _ap3(ident[:], [[ident[:].ap[0][0], H], [1, H]])
                    nc.tensor.matmul(
                        tp[:],
                        lhsT=_ap3(h_fm[:], [[h_fm[:].ap[0][0], H], [1, 128]],
                                  h_fm[:].offset + w * 128),
                        rhs=ident64,
                        start=True, stop=True, skip_group_check=True)
                    nc.scalar.copy(h_nm[:, w * H:(w + 1) * H], tp[:])
                for blk in range(3):
                    SG = spool.tile([128, CALLCH * 128], BF16, tag="S",
                                    name=f"SG{g}_{blk}")
                    iob = _ap3(iotaP[:], [[iotaP[:].ap[0][0], 128],
                                          [0, nb], [1, 128]],
                               iotaP[:].offset + blk * 128)
                    nc.vector.tensor_tensor(
                        _ap3(SG[:], [[SG[:].ap[0][0], 128], [128, nb], [1, 128]]),
                        iob, sc_bc(glocal, wlo, nb, 128), op=ALU.is_equal)
                    for i in range(nb):
                        w = wlo + i
                        mm = nc.tensor.matmul(
                            pps[:, blk * H:(blk + 1) * H],
                            lhsT=SG[:, i * 128:(i + 1) * 128],
                            rhs=h_nm[:, w * H:(w + 1) * H],
                            start=(blk == 0 and w == 0),
                            stop=(blk == 2 and w == W - 1),
                            skip_group_check=True)
                        if blk == 0 and w == 0:
                            pool_state["start"] = mm
                        else:
                            bass._add_dep_helper(
                                mm.ins, pool_state["start"].ins,
                                sync=False, reason="psum order")

            # ================= schedule =================
            with tc.tile_pool(name="xfm", bufs=2) as xpool:
                conv1(xpool)
            if debug:
                nc.sync.dma_start(dbg["tbl1"].ap(), tbl[0].ap())
            for l in range(1, 6):
                if l < 5:
                    aggregate(l, make_transform(l))
                else:
                    aggregate(l, pool_group)

            ppsb = tpool.tile([128, 3 * H], F32, tag="ppsb")
            nc.vector.tensor_copy(ppsb[:], pool_state["pps"][:])
            nc.sync.dma_start(out_e.ap(), ppsb[:])


def _ensure_ntff_hook():
    import sys as _sys, types as _types
    try:
        from antenv.axon_hooks import get_axon_ntff_profile_hook  # noqa
        return
    except ImportError:
        pass
    try:
        import antenv
        mod = _types.ModuleType("antenv.axon_hooks")
        _state = {"hook": None}
        mod.set_axon_ntff_profile_hook = lambda h: _state.__setitem__("hook", h)
        mod.get_axon_ntff_profile_hook = lambda: _state["hook"]
        _sys.modules["antenv.axon_hooks"] = mod
        antenv.axon_hooks = mod
        from trn_agent_boot.trn_boot import _ntff_profile_via_ctypes
        mod.set_axon_ntff_profile_hook(
            _ntff_profile_via_ctypes("/opt/axon/libaxon_pjrt.so"))
    except Exception:
        pass


def run(inputs, C=8, G=1000, trace=False, debug=False):
    if trace:
        _ensure_ntff_hook()
    meta, in_maps = prep(
        inputs["x"], inputs["W1"], inputs["b1"], inputs["W2"], inputs["b2"],
        inputs["Wl"], inputs["bl"], inputs["edge_index"], inputs["batch"],
        C=C, G=G)
    nc = bacc.Bacc("TRN2", target_bir_lowering=False, debug=False,
                   num_devices=C, num_swdge_queues=NQ)
    build(nc, meta, debug=debug)
    nc.compile()
    from concourse.bass_utils import run_bass_kernel_spmd
    res = run_bass_kernel_spmd(nc, in_maps, core_ids=list(range(C)),
                               trace=trace)
    parts = [res.results[c]["out"] for c in range(C)]
    out = host_finish(meta, parts, inputs, C, G)
    return out, res


def kernel(**inputs):
    out, _ = run(inputs)
    return out


def host_finish(meta, parts, inputs, C, G):
    H = meta["H"]
    pooled = np.zeros(((meta["BLK"] + 3) * 128, H), np.float32)
    for c in range(C):
        part = np.asarray(parts[c], np.float32)
        base = meta["wk"][c] * 128
        for b in range(3):
            pooled[base + b * 128: base + (b + 1) * 128] += \
                part[:, b * H:(b + 1) * H]
    counts = np.bincount(np.asarray(inputs["batch"], np.int64),
                         minlength=G).astype(np.float32)
    pooledG = pooled[:G] / np.maximum(counts, 1.0)[:, None]
    Wl = np.asarray(inputs["Wl"], np.float32).reshape(H, -1)
    bl = np.asarray(inputs["bl"], np.float32)
    return (pooledG @ Wl + bl).astype(np.float32)


if __name__ == "__main__":
    import time
    sys.path.insert(0, "/root/problem")
    mode = sys.argv[1] if len(sys.argv) > 1 else "sim"
    import jax
    cpu = jax.devices("cpu")[0]
    with jax.default_device(cpu):
        import reference
        inputs = {k: np.asarray(v) for k, v in reference.setup_inputs().items()}
        expected = np.asarray(reference.reference(**inputs))
    if mode == "sim":
        meta, in_maps = prep(
            inputs["x"], inputs["W1"], inputs["b1"], inputs["W2"], inputs["b2"],
            inputs["Wl"], inputs["bl"], inputs["edge_index"], inputs["batch"],
            C=8, G=1000)
        print("NCHUNK", meta["NCHUNK"], "calls/layer", len(meta["calls"]))
        parts = sim_global(meta, in_maps, inputs, 8, 1000)
        out = host_finish(meta, parts, inputs, 8, 1000)
    else:
        t1 = time.time()
        out, res = run(inputs, C=8, G=1000, trace=(mode == "hw"),
                       debug=(mode == "dbg"))
        print(f"kernel total: {time.time()-t1:.1f}s", flush=True)
        if mode == "hw":
            print(f"HW exec time: {res.exec_time_ns} ns")
    err = np.abs(out - expected).max()
    rel = err / max(np.abs(expected).max(), 1e-9)
    print(f"max abs err {err:.6g}")
    print(f"Relative error: {rel:.6g}")
